# revision 1
# baseline (speedup 1.0000x reference)
"""Trainium2 Bass kernel: 3D affine spatial transformer (affine_grid +
trilinear grid_sample, align_corners=True, zeros padding).

Data parallel: one sample per NeuronCore (8 cores). Per core, output is
processed in 10-wide w-chunk "tasks". One indirect DMA per task-group
(40 tasks x 3 z-slots = 120 partition streams) fetches, per (task, jz),
a contiguous 1180-element stream from a zero-padded channel-interleaved
copy (PV) of src in DRAM, covering the task's (y-band x x-window x 2ch)
window. Exact trilinear weights are hat functions relu(1-|t|) evaluated
densely over the (jz, jy, s) window; a PE matmul with a block-selection
matrix collapses the jz slots and emits per-task outputs.
"""

import numpy as np

import concourse.bass as bass
import concourse.bacc as bacc
import concourse.mybir as mybir
from concourse import tile
from concourse.bass import AP, DynSlice
from concourse.bass_utils import run_bass_kernel_spmd

F32 = mybir.dt.float32
I32 = mybir.dt.int32
AO = mybir.AluOpType

N, C, D, H, W = 8, 2, 96, 160, 160
SRCEL = D * H * W
OUTEL = C * SRCEL

L = 10
KZ, KY, KX = 3, 4, 5
MZ, MY, MX = 4, 4, 16
ZP, YP, XP = D + 2 * MZ, H + 3 * MY, W + 2 * MX      # 104, 172, 192
ZSTR = YP * XP * C                                    # 66048
YSTR = XP * C                                         # 384
PVN = ZP * ZSTR                                       # 6868992
STREAM = (KY - 1) * YSTR + ((L - 1) + KX) * C         # 1180
TPG = 40
NTASK = D * H * (W // L)                              # 245760
NG = NTASK // TPG                                     # 6144
NB = 4
NBLK = NG // NB                                       # 1536
ZB_MAX, YB_MAX, XB_MAX = float(ZP - KZ), 167.0, 178.0

_CACHE = {}


def _build_program():
    P = 128
    nc = bacc.Bacc(None, target_bir_lowering=False)
    src = nc.declare_dram_parameter("src", [C, SRCEL], F32, isOutput=False)
    theta = nc.declare_dram_parameter("theta", [1, 12], F32, isOutput=False)
    out = nc.declare_dram_parameter("out", [1, OUTEL], F32, isOutput=True)
    pv = nc.dram_tensor("pv", [PVN, 1], F32)

    def fb(apx, pairs, extra_off=0):
        """clone AP keeping partition pair, replacing free pairs"""
        return AP(apx.tensor, apx.offset + extra_off,
                  [list(apx.ap[0])] + [list(p) for p in pairs])

    with tile.TileContext(nc) as tc:
        with (
            tc.tile_pool(name="big", bufs=1) as big,
            tc.tile_pool(name="pp", bufs=2, space="PSUM") as pp,
        ):
            # ---------- phase 0: zero-fill PV ----------
            pre_ctx = tc.tile_pool(name="pre", bufs=2)
            pre = pre_ctx.__enter__()
            zt = pre.tile([P, 4096], F32)
            nc.vector.memset(zt[:], 0.0)
            CH = P * 4096
            nfull = PVN // CH
            for i in range(nfull):
                nc.sync.dma_start(out=AP(pv[:].tensor, i * CH, [[1, CH]]),
                                  in_=zt[:])
            rem = PVN - nfull * CH          # 53248 = 128*416
            nc.sync.dma_start(out=AP(pv[:].tensor, nfull * CH, [[1, rem]]),
                              in_=zt[:, :rem // P])

            # ---------- phase 1: build PV (channel interleave) ----------
            for z in range(D):
                for yc in range(2):
                    s0 = pre.tile([80, W], F32, tag="s0")
                    s1 = pre.tile([80, W], F32, tag="s1")
                    off = z * H * W + yc * 80 * W
                    nc.sync.dma_start(out=s0[:], in_=AP(
                        src[:].tensor, off, [[W, 80], [1, W]]))
                    nc.sync.dma_start(out=s1[:], in_=AP(
                        src[:].tensor, SRCEL + off, [[W, 80], [1, W]]))
                    il = pre.tile([80, W * C], F32, tag="il")
                    nc.vector.tensor_copy(out=fb(il[:], [[2, W]]), in_=s0[:])
                    nc.vector.tensor_copy(out=fb(il[:], [[2, W]], 1), in_=s1[:])
                    dst_off = ((z + MZ) * ZSTR + (MY + yc * 80) * YSTR
                               + MX * C)
                    nc.sync.dma_start(
                        out=AP(pv[:].tensor, dst_off, [[YSTR, 80], [1, W * C]]),
                        in_=il[:])

            # ---------- phase 2: scalars & constants ----------
            th0 = big.tile([P, 12], F32)
            nc.sync.dma_start(out=th0[:1, :], in_=theta[:])
            ones1 = big.tile([1, P], F32)
            nc.vector.memset(ones1[:], 1.0)
            thps = pp.tile([P, 12], F32, tag="thps")
            nc.tensor.matmul(out=thps[:], lhsT=ones1[:], rhs=th0[:1, :],
                             start=True, stop=True)
            thb = big.tile([P, 12], F32)
            nc.vector.tensor_copy(out=thb[:], in_=thps[:])

            def thc(j):
                return thb[:, j:j + 1]

            sc = big.tile([P, 20], F32)
            (AX, BX, CXc, OX, AY, BY, CYc, OY, AZ, BZ, CZc, OZ,
             AXM1, SP_X, SP_Y, SP_Z) = range(16)

            def scc(j):
                return sc[:, j:j + 1]

            r = 159.0 / 95.0

            def row(dsti, srci, mulc, a_i, b_i, c_i, osc, oadd):
                # A,B,C,O for one axis. srci = first theta idx of the row
                nc.vector.tensor_copy(out=scc(dsti[0]), in_=thc(srci))
                nc.vector.tensor_copy(out=scc(dsti[1]), in_=thc(srci + 1))
                nc.vector.tensor_scalar_mul(out=scc(dsti[2]),
                                            in0=thc(srci + 2), scalar1=mulc)
                tmp = pre.tile([P, 1], F32, tag="sctmp")
                nc.vector.tensor_tensor(out=tmp[:], in0=thc(srci + 3),
                                        in1=thc(srci), op=AO.subtract)
                nc.vector.tensor_tensor(out=tmp[:], in0=tmp[:],
                                        in1=thc(srci + 1), op=AO.subtract)
                nc.vector.tensor_tensor(out=tmp[:], in0=tmp[:],
                                        in1=thc(srci + 2), op=AO.subtract)
                nc.vector.tensor_scalar(out=scc(dsti[3]), in0=tmp[:],
                                        scalar1=osc, scalar2=osc + oadd,
                                        op0=AO.mult, op1=AO.add)

            # x: A=t00 B=t01 C=t02*r O=79.5*(1+t03-t00-t01-t02)+MX
            row((AX, BX, CXc, OX), 0, r, None, None, None, 79.5, float(MX))
            row((AY, BY, CYc, OY), 4, r, None, None, None, 79.5, float(MY))
            # z: A=t20/r B=t21/r C=t22 O=47.5*(...)+MZ
            nc.vector.tensor_scalar_mul(out=scc(AZ), in0=thc(8), scalar1=1.0 / r)
            nc.vector.tensor_scalar_mul(out=scc(BZ), in0=thc(9), scalar1=1.0 / r)
            nc.vector.tensor_copy(out=scc(CZc), in_=thc(10))
            tmp = pre.tile([P, 1], F32, tag="sctmp2")
            nc.vector.tensor_tensor(out=tmp[:], in0=thc(11), in1=thc(8),
                                    op=AO.subtract)
            nc.vector.tensor_tensor(out=tmp[:], in0=tmp[:], in1=thc(9),
                                    op=AO.subtract)
            nc.vector.tensor_tensor(out=tmp[:], in0=tmp[:], in1=thc(10),
                                    op=AO.subtract)
            nc.vector.tensor_scalar(out=scc(OZ), in0=tmp[:], scalar1=47.5,
                                    scalar2=47.5 + MZ, op0=AO.mult, op1=AO.add)
            nc.vector.tensor_scalar_add(out=scc(AXM1), in0=scc(AX),
                                        scalar1=-1.0)
            nc.vector.tensor_scalar_mul(out=scc(SP_X), in0=scc(AXM1),
                                        scalar1=float(L - 1))
            nc.vector.tensor_scalar_mul(out=scc(SP_Y), in0=scc(AY),
                                        scalar1=float(L - 1))
            nc.vector.tensor_scalar_mul(out=scc(SP_Z), in0=scc(AZ),
                                        scalar1=float(L - 1))

            # per-partition: p, tl=floor(p/3), jz=p-3tl
            pidi = big.tile([P, 1], I32)
            nc.gpsimd.iota(pidi[:], [[0, 1]], base=0, channel_multiplier=1)
            pid = big.tile([P, 1], F32)
            nc.vector.tensor_copy(out=pid[:], in_=pidi[:])
            tl = big.tile([P, 1], F32)
            tli = big.tile([P, 1], I32)
            nc.vector.tensor_scalar(out=tl[:], in0=pid[:], scalar1=-1.0,
                                    scalar2=1.0 / 3.0, op0=AO.add, op1=AO.mult)
            nc.vector.tensor_copy(out=tli[:], in_=tl[:])
            nc.vector.tensor_copy(out=tl[:], in_=tli[:])
            jz = big.tile([P, 1], F32)
            nc.vector.scalar_tensor_tensor(out=jz[:], in0=tl[:], scalar=-3.0,
                                           in1=pid[:], op0=AO.mult, op1=AO.add)
            jzoff = big.tile([P, 1], F32)
            nc.vector.tensor_scalar_mul(out=jzoff[:], in0=jz[:],
                                        scalar1=float(ZSTR))

            # Sel [128, TPG] = (tl == m), zero for idle partitions (tl>=40)
            mio = big.tile([P, TPG], I32)
            nc.gpsimd.iota(mio[:], [[1, TPG]], base=0, channel_multiplier=0)
            miof = big.tile([P, TPG], F32)
            nc.vector.tensor_copy(out=miof[:], in_=mio[:])
            sel = big.tile([P, TPG], F32)
            nc.vector.tensor_tensor(out=sel[:], in0=fb(tl[:], [[0, TPG]]),
                                    in1=miof[:], op=AO.is_equal)

            # iotas
            def iotaf(n, tag):
                ti_ = big.tile([P, n], I32, tag=tag + "i")
                nc.gpsimd.iota(ti_[:], [[1, n]], base=0, channel_multiplier=0)
                tf_ = big.tile([P, n], F32, tag=tag + "f")
                nc.vector.tensor_copy(out=tf_[:], in_=ti_[:])
                return tf_

            wlf = iotaf(L, "wl")
            jyf = iotaf(KY, "jy")
            sxf = iotaf(KX, "sx")

            # global field templates (computed once):
            # zwlG[wl] = Az*wl - jz ; ywlG[jy,wl] = Ay*wl - jy ;
            # xwlG[s,wl] = (Ax-1)*wl - s
            zwlG = big.tile([P, L], F32)
            nc.vector.scalar_tensor_tensor(
                out=zwlG[:], in0=wlf[:], scalar=scc(AZ),
                in1=fb(jz[:], [[0, L]]), op0=AO.mult, op1=AO.subtract)
            ywlG = big.tile([P, KY, L], F32)
            nc.vector.scalar_tensor_tensor(
                out=ywlG[:], in0=fb(wlf[:], [[0, KY], [1, L]]),
                scalar=scc(AY), in1=fb(jyf[:], [[1, KY], [0, L]]),
                op0=AO.mult, op1=AO.subtract)
            xwlG = big.tile([P, KX, L], F32)
            nc.vector.scalar_tensor_tensor(
                out=xwlG[:], in0=fb(wlf[:], [[0, KX], [1, L]]),
                scalar=scc(AXM1), in1=fb(sxf[:], [[1, KX], [0, L]]),
                op0=AO.mult, op1=AO.subtract)

            # ---------- phase 3: per-task residuals + stream indices ----
            idxT = big.tile([P, NG], I32)
            rzT = big.tile([P, NG], F32)
            ryT = big.tile([P, NG], F32)
            rxT = big.tile([P, NG], F32)

            CHG = 512
            for c0 in range(0, NG, CHG):
                n = CHG
                gi = pre.tile([P, n], I32, tag="gi")
                nc.gpsimd.iota(gi[:], [[TPG, n]], base=c0 * TPG,
                               channel_multiplier=0)
                tt = pre.tile([P, n], F32, tag="tt")
                nc.vector.tensor_copy(out=tt[:], in_=gi[:])
                nc.vector.tensor_scalar_add(out=tt[:], in0=tt[:],
                                            scalar1=tl[:])
                ti = pre.tile([P, n], I32, tag="ti")

                def fdiv(outt, int_, dv):
                    nc.vector.tensor_scalar(out=outt, in0=int_,
                                            scalar1=-(dv - 1.0) / 2.0,
                                            scalar2=1.0 / dv, op0=AO.add,
                                            op1=AO.mult)
                    nc.vector.tensor_copy(out=ti[:], in_=outt)
                    nc.vector.tensor_copy(out=outt, in_=ti[:])

                dd = pre.tile([P, n], F32, tag="dd")
                fdiv(dd[:], tt[:], 2560.0)
                rem_ = pre.tile([P, n], F32, tag="rem")
                nc.vector.scalar_tensor_tensor(out=rem_[:], in0=dd[:],
                                               scalar=-2560.0, in1=tt[:],
                                               op0=AO.mult, op1=AO.add)
                hh = pre.tile([P, n], F32, tag="hh")
                fdiv(hh[:], rem_[:], 16.0)
                w0 = pre.tile([P, n], F32, tag="w0")
                nc.vector.scalar_tensor_tensor(out=w0[:], in0=hh[:],
                                               scalar=-16.0, in1=rem_[:],
                                               op0=AO.mult, op1=AO.add)
                nc.vector.tensor_scalar_mul(out=w0[:], in0=w0[:],
                                            scalar1=float(L))

                acci = pre.tile([P, n], F32, tag="acci")

                def base_resid(ai, bi, ci, oi, spi, bmax, resT, strd, first):
                    cin = pre.tile([P, n], F32, tag="cin")
                    nc.vector.tensor_scalar_mul(out=cin[:], in0=w0[:],
                                                scalar1=scc(ai))
                    nc.vector.scalar_tensor_tensor(
                        out=cin[:], in0=hh[:], scalar=scc(bi), in1=cin[:],
                        op0=AO.mult, op1=AO.add)
                    nc.vector.scalar_tensor_tensor(
                        out=cin[:], in0=dd[:], scalar=scc(ci), in1=cin[:],
                        op0=AO.mult, op1=AO.add)
                    nc.vector.tensor_scalar_add(out=cin[:], in0=cin[:],
                                                scalar1=scc(oi))
                    c9 = pre.tile([P, n], F32, tag="c9")
                    nc.vector.tensor_scalar_add(out=c9[:], in0=cin[:],
                                                scalar1=scc(spi))
                    nc.vector.tensor_tensor(out=c9[:], in0=c9[:], in1=cin[:],
                                            op=AO.min)
                    cb = pre.tile([P, n], F32, tag="cb")
                    nc.vector.tensor_scalar_add(out=cb[:], in0=c9[:],
                                                scalar1=-0.499999)
                    nc.vector.tensor_copy(out=ti[:], in_=cb[:])
                    nc.vector.tensor_copy(out=cb[:], in_=ti[:])
                    nc.vector.tensor_scalar_max(out=cb[:], in0=cb[:],
                                                scalar1=0.0)
                    nc.vector.tensor_scalar_min(out=cb[:], in0=cb[:],
                                                scalar1=bmax)
                    nc.vector.tensor_tensor(out=resT[:, c0:c0 + n],
                                            in0=cin[:], in1=cb[:],
                                            op=AO.subtract)
                    if first:
                        nc.vector.tensor_scalar_mul(out=acci[:], in0=cb[:],
                                                    scalar1=float(strd))
                    else:
                        nc.vector.scalar_tensor_tensor(
                            out=acci[:], in0=cb[:], scalar=float(strd),
                            in1=acci[:], op0=AO.mult, op1=AO.add)

                base_resid(AZ, BZ, CZc, OZ, SP_Z, ZB_MAX, rzT, ZSTR, True)
                base_resid(AY, BY, CYc, OY, SP_Y, YB_MAX, ryT, YSTR, False)
                base_resid(AX, BX, CXc, OX, SP_X, XB_MAX, rxT, C, False)
                nc.vector.tensor_scalar_add(out=acci[:], in0=acci[:],
                                            scalar1=jzoff[:])
                nc.vector.tensor_copy(out=idxT[:, c0:c0 + n], in_=acci[:])

            # ---------- phase 4: main loop ----------
            pre_ctx.__exit__(None, None, None)
            gtp_ctx = tc.tile_pool(name="gtp", bufs=1)
            gtp = gtp_ctx.__enter__()
            work_ctx = tc.tile_pool(name="work", bufs=2)
            work = work_ctx.__enter__()
            UNR = 4
            with tc.For_i(0, NBLK // UNR, 1, staggered_reset=True) as ib:
              gts = []
              for u_ in range(UNR):
                  idxfix = gtp.tile([P, NB], I32, tag=f"idxfix{u_}")
                  nc.vector.tensor_copy(
                      out=idxfix[:],
                      in_=idxT[:, DynSlice((ib * UNR + u_) * NB, NB)])
                  gt = gtp.tile([P, NB * STREAM], F32, tag=f"gt{u_}")
                  for j in range(NB):
                      nc.gpsimd.indirect_dma_start(
                          out=gt[:, j * STREAM:(j + 1) * STREAM],
                          out_offset=None,
                          in_=pv[:],
                          in_offset=bass.IndirectOffsetOnAxis(
                              ap=idxfix[:, j:j + 1], axis=0))
                  gts.append(gt)
              for u_ in range(UNR):
                  gt = gts[u_]
                  rzs = rzT[:, DynSlice((ib * UNR + u_) * NB, NB)]
                  rys = ryT[:, DynSlice((ib * UNR + u_) * NB, NB)]
                  rxs = rxT[:, DynSlice((ib * UNR + u_) * NB, NB)]

                  zf = work.tile([P, NB, L], F32, tag="zf")
                  nc.vector.tensor_tensor(
                      out=zf[:], in0=fb(zwlG[:], [[0, NB], [1, L]]),
                      in1=fb(rzs, [[1, NB], [0, L]]), op=AO.add)
                  hz = work.tile([P, NB, L], F32, tag="hz")
                  nc.scalar.activation(hz[:], zf[:],
                                       mybir.ActivationFunctionType.Abs)
                  nc.scalar.activation(hz[:], hz[:],
                                       mybir.ActivationFunctionType.Relu,
                                       bias=1.0, scale=-1.0)
                  yf = work.tile([P, NB, KY * L], F32, tag="yf")
                  nc.vector.tensor_tensor(
                      out=yf[:], in0=fb(ywlG[:], [[0, NB], [1, KY * L]]),
                      in1=fb(rys, [[1, NB], [0, KY * L]]), op=AO.add)
                  hy = work.tile([P, NB, KY * L], F32, tag="hy")
                  nc.scalar.activation(hy[:], yf[:],
                                       mybir.ActivationFunctionType.Abs)
                  nc.scalar.activation(hy[:], hy[:],
                                       mybir.ActivationFunctionType.Relu,
                                       bias=1.0, scale=-1.0)
                  xf = work.tile([P, NB, KX * L], F32, tag="xf")
                  nc.vector.tensor_tensor(
                      out=xf[:], in0=fb(xwlG[:], [[0, NB], [1, KX * L]]),
                      in1=fb(rxs, [[1, NB], [0, KX * L]]), op=AO.add)
                  gx = work.tile([P, NB, KX * L], F32, tag="gx")
                  nc.scalar.activation(gx[:], xf[:],
                                       mybir.ActivationFunctionType.Abs)
                  nc.scalar.activation(gx[:], gx[:],
                                       mybir.ActivationFunctionType.Relu,
                                       bias=1.0, scale=-1.0)
                  nc.vector.tensor_tensor(
                      out=gx[:], in0=gx[:],
                      in1=fb(hz[:], [[L, NB], [0, KX], [1, L]]), op=AO.mult)
                  w3 = work.tile([P, NB, KY, KX, L], F32, tag="w3")
                  for jyv in range(KY):
                      w3s = AP(w3[:].tensor, w3[:].offset + jyv * KX * L,
                               [list(w3[:].ap[0]), [KY * KX * L, NB],
                                [L, KX], [1, L]])
                      nc.vector.tensor_tensor(
                          out=w3s,
                          in0=fb(gx[:], [[KX * L, NB], [L, KX], [1, L]]),
                          in1=AP(hy[:].tensor, hy[:].offset + jyv * L,
                                 [list(hy[:].ap[0]), [KY * L, NB], [0, KX],
                                  [1, L]]),
                          op=AO.mult)

                  rt = work.tile([P, C, NB, L], F32, tag="rt")
                  for ch in range(C):
                      prod = work.tile([P, NB, L, KY, KX], F32, tag="pr")
                      for jyv in range(KY):
                          ps_ = AP(prod[:].tensor, prod[:].offset + jyv * KX,
                                   [list(prod[:].ap[0]), [L * KY * KX, NB],
                                    [KY * KX, L], [1, KX]])
                          dap = AP(gt[:].tensor,
                                   gt[:].offset + ch + jyv * YSTR,
                                   [list(gt[:].ap[0]), [STREAM, NB], [C, L],
                                    [C, KX]])
                          wap = AP(w3[:].tensor, w3[:].offset + jyv * KX * L,
                                   [list(w3[:].ap[0]), [KY * KX * L, NB],
                                    [1, L], [L, KX]])
                          nc.vector.tensor_tensor(out=ps_, in0=dap, in1=wap,
                                                  op=AO.mult)
                      nc.vector.tensor_reduce(
                          out=rt[:, ch, :, :].rearrange("p a b -> p (a b)"),
                          in_=prod[:].rearrange("p g w a b -> p (g w) (a b)"),
                          op=AO.add, axis=mybir.AxisListType.X)

                  ps = pp.tile([TPG, C * NB * L], F32, tag="ps")
                  nc.tensor.matmul(
                      out=ps[:], lhsT=sel[:],
                      rhs=rt[:].rearrange("p c g w -> p (c g w)"),
                      start=True, stop=True)
                  stg = work.tile([TPG, C, NB, L], F32, tag="stg")
                  nc.vector.tensor_copy(
                      out=stg[:].rearrange("p c g w -> p (c g w)"), in_=ps[:])
                  for ch in range(C):
                      dsl = out[0, DynSlice((ib * UNR + u_) * (NB * TPG * L)
                                            + ch * SRCEL, NB * TPG * L)]
                      dst = AP(dsl.tensor, dsl.offset,
                               [[L, TPG], [TPG * L, NB], [1, L]])
                      sap = stg[:]
                      srcap = AP(sap.tensor, sap.offset + ch * NB * L,
                                 [list(sap.ap[0]), [L, NB], [1, L]])
                      nc.sync.dma_start(out=dst, in_=srcap)
            work_ctx.__exit__(None, None, None)
            gtp_ctx.__exit__(None, None, None)

    nc.compile()
    return nc


def kernel(src, theta):
    if "prog" not in _CACHE:
        _CACHE["prog"] = _build_program()
    nc = _CACHE["prog"]
    in_maps = []
    for i in range(N):
        in_maps.append({
            "src": np.ascontiguousarray(src[i].reshape(C, SRCEL),
                                        dtype=np.float32),
            "theta": np.ascontiguousarray(theta[i].reshape(1, 12),
                                          dtype=np.float32),
        })
    res = run_bass_kernel_spmd(nc, in_maps, core_ids=list(range(N)))
    o = np.empty((N, C, D, H, W), dtype=np.float32)
    for i in range(N):
        o[i] = res.results[i]["out"].reshape(C, D, H, W)
    return o



# revision 4
# speedup vs baseline: 1.6145x; 1.6145x over previous
"""Trainium2 Bass kernel v2: 3D affine spatial transformer.

Data parallel: one sample per NeuronCore. Per core:
 - PV: strip-padded bf16 copy of src in DRAM, layout
   [strip 10][z 100][y 168][c 2][x 32]; strips are 32-wide x-windows every
   19 PV-x columns so any 13-wide x-window lives in one strip.
 - Output processed in L=10-wide w-chunk "tasks"; per task a base
   (strip, z0, y0, xloc) is computed on-device from theta; per (task, jz)
   ONE contiguous 238-elem bf16 stream covers KY=4 y-rows x 2ch x 13 x.
 - Gathers batched: one indirect DMA per block = 120 partitions
   (40 tasks x 3 jz) x G=16 streams = 1920 descriptors.
 - Sampling: dense hat-weight taps, separable x-stage (KX=4 shifts) then
   y-stage (KY=4) on DVE in bf16; jz collapse via PE matmul with a 0/1
   selection matrix into PSUM f32.
"""

import numpy as np

import concourse.bass as bass
import concourse.bacc as bacc
import concourse.mybir as mybir
from concourse import tile
from concourse.bass import AP, DynSlice
from concourse.bass_utils import run_bass_kernel_spmd

F32 = mybir.dt.float32
F16 = mybir.dt.float16
BF16 = mybir.dt.bfloat16
I32 = mybir.dt.int32
AO = mybir.AluOpType
ACTF = mybir.ActivationFunctionType

N, C, D, H, W = 8, 2, 96, 160, 160
SRCEL = D * H * W
OUTEL = C * SRCEL

L = 10
KZ, KY, KX = 3, 4, 5
WX = (L - 1) + KX                 # 13
MZ, MY, MX = 2, 4, 16
ZP, YP = D + 2 * MZ, H + 2 * MY   # 100, 168
NSTRIP, S0, SW = 10, 19, 32
RL = SW * C                       # 64 elems per (y) row group
ZSTR = YP * RL                    # 10752
SSTR = ZP * ZSTR                  # 1075200
PVN = NSTRIP * SSTR + 64          # + pad for stream overrun
STRM = 238
ZB_MAX, YB_MAX, XB_MAX = 97.0, 164.0, 178.0

TPG = 40                          # tasks per group (128 partitions: 40x3)
NP = TPG * KZ                     # 120 active partitions
G = 16                            # groups batched per indirect DMA
NT = D * H * (W // L)             # 245760 tasks
NCOL = NT // TPG                  # 6144 idx columns
NBLK = NCOL // G                  # 384 blocks
UNR = 2

_CACHE = {}


def _build_program():
    P = 128
    nc = bacc.Bacc(None, target_bir_lowering=False)
    src = nc.declare_dram_parameter("src", [C, SRCEL], F32, isOutput=False)
    theta = nc.declare_dram_parameter("theta", [1, 12], F32, isOutput=False)
    out = nc.declare_dram_parameter("out", [1, OUTEL], F32, isOutput=True)
    pv = nc.dram_tensor("pv", [PVN, 1], BF16)

    def fb(apx, pairs, extra_off=0):
        """clone AP keeping partition pair, replacing free pairs"""
        return AP(apx.tensor, apx.offset + extra_off,
                  [list(apx.ap[0])] + [list(p) for p in pairs])

    with tile.TileContext(nc) as tc:
        with (
            tc.tile_pool(name="big", bufs=1) as big,
            tc.tile_pool(name="pp", bufs=2, space="PSUM") as pp,
        ):
            # ---------- phase 0: zero-fill PV ----------
            pre_ctx = tc.tile_pool(name="pre", bufs=2)
            pre = pre_ctx.__enter__()
            zt = pre.tile([P, 4096], BF16)
            nc.vector.memset(zt[:], 0.0)
            CH = P * 4096
            nfull = PVN // CH
            for i in range(nfull):
                nc.sync.dma_start(out=AP(pv[:].tensor, i * CH, [[1, CH]]),
                                  in_=zt[:])
            rem = PVN - nfull * CH
            remp = rem // P
            nc.sync.dma_start(
                out=AP(pv[:].tensor, nfull * CH, [[1, remp * P]]),
                in_=zt[:, :remp])
            rem2 = rem - remp * P
            if rem2:
                nc.sync.dma_start(
                    out=AP(pv[:].tensor, PVN - rem2, [[1, rem2]]),
                    in_=zt[:1, :rem2])

            # ---------- phase 1: build PV ----------
            # per (zb of NZB z-slices, yb of 80 rows):
            #   load [80, NZB*160] f32 per c; assemble T [80, z c s x] bf16;
            #   DMA per strip [80, z c x].
            NZB = 8
            for zb in range(D // NZB):
                for yb in range(2):
                    ld = []
                    for c in range(C):
                        lt = pre.tile([80, NZB * W], F32, tag=f"ld{c}")
                        nc.sync.dma_start(
                            out=lt[:],
                            in_=AP(src[:].tensor,
                                   c * SRCEL + zb * NZB * H * W + yb * 80 * W,
                                   [[W, 80], [H * W, NZB], [1, W]]))
                        ld.append(lt)
                    # T layout [80][z NZB][s 10][c 2][x 32]
                    TZ = NSTRIP * C * SW       # 640, z-stride in T
                    T = pre.tile([80, NZB * TZ], BF16, tag="T")
                    nc.vector.memset(T[:], 0.0)
                    for c in range(C):
                        # interior strips 1..7: src x = 19s-16 .. +32
                        nc.vector.tensor_copy(
                            out=fb(T[:], [[TZ, NZB], [C * SW, 7], [1, SW]],
                                   extra_off=1 * C * SW + c * SW),
                            in_=fb(ld[c][:], [[W, NZB], [S0, 7], [1, SW]],
                                   extra_off=3))
                        # strip 0: src x [-16,16) -> valid [0,16) at xl 16..32
                        nc.vector.tensor_copy(
                            out=fb(T[:], [[TZ, NZB], [1, 16]],
                                   extra_off=c * SW + 16),
                            in_=fb(ld[c][:], [[W, NZB], [1, 16]]))
                        # strip 8: src x [136,168) -> valid 24 at xl 0..24
                        nc.vector.tensor_copy(
                            out=fb(T[:], [[TZ, NZB], [1, 24]],
                                   extra_off=8 * C * SW + c * SW),
                            in_=fb(ld[c][:], [[W, NZB], [1, 24]],
                                   extra_off=136))
                        # strip 9: src x [155,187) -> valid 5 at xl 0..5
                        nc.vector.tensor_copy(
                            out=fb(T[:], [[TZ, NZB], [1, 5]],
                                   extra_off=9 * C * SW + c * SW),
                            in_=fb(ld[c][:], [[W, NZB], [1, 5]],
                                   extra_off=155))
                    for s in range(NSTRIP):
                        dst_off = (s * SSTR + (MZ + zb * NZB) * ZSTR
                                   + (MY + yb * 80) * RL)
                        nc.sync.dma_start(
                            out=AP(pv[:].tensor, dst_off,
                                   [[RL, 80], [ZSTR, NZB], [1, C * SW]]),
                            in_=fb(T[:], [[TZ, NZB], [1, C * SW]],
                                   extra_off=s * C * SW))

            # ---------- phase 2: scalars & constants ----------
            th0 = big.tile([P, 12], F32)
            nc.sync.dma_start(out=th0[:1, :], in_=theta[:])
            ones1 = big.tile([1, P], F32)
            nc.vector.memset(ones1[:], 1.0)
            thps = pp.tile([P, 12], F32, tag="thps")
            nc.tensor.matmul(out=thps[:], lhsT=ones1[:], rhs=th0[:1, :],
                             start=True, stop=True)
            thb = big.tile([P, 12], F32)
            nc.vector.tensor_copy(out=thb[:], in_=thps[:])

            def thc(j):
                return thb[:, j:j + 1]

            sc = big.tile([P, 20], F32)
            (AXi, BXi, CXi, OXi, AYi, BYi, CYi, OYi, AZi, BZi, CZi, OZi,
             AXM1, SPXi, SPYi, SPZi) = range(16)

            def scc(j):
                return sc[:, j:j + 1]

            r = 159.0 / 95.0

            def row(dsti, srci, mulc, osc, oadd):
                nc.vector.tensor_copy(out=scc(dsti[0]), in_=thc(srci))
                nc.vector.tensor_copy(out=scc(dsti[1]), in_=thc(srci + 1))
                nc.vector.tensor_scalar_mul(out=scc(dsti[2]),
                                            in0=thc(srci + 2), scalar1=mulc)
                tmp = pre.tile([P, 1], F32, tag="sctmp")
                nc.vector.tensor_tensor(out=tmp[:], in0=thc(srci + 3),
                                        in1=thc(srci), op=AO.subtract)
                nc.vector.tensor_tensor(out=tmp[:], in0=tmp[:],
                                        in1=thc(srci + 1), op=AO.subtract)
                nc.vector.tensor_tensor(out=tmp[:], in0=tmp[:],
                                        in1=thc(srci + 2), op=AO.subtract)
                nc.vector.tensor_scalar(out=scc(dsti[3]), in0=tmp[:],
                                        scalar1=osc, scalar2=osc + oadd,
                                        op0=AO.mult, op1=AO.add)

            row((AXi, BXi, CXi, OXi), 0, r, 79.5, float(MX))
            row((AYi, BYi, CYi, OYi), 4, r, 79.5, float(MY))
            nc.vector.tensor_scalar_mul(out=scc(AZi), in0=thc(8),
                                        scalar1=1.0 / r)
            nc.vector.tensor_scalar_mul(out=scc(BZi), in0=thc(9),
                                        scalar1=1.0 / r)
            nc.vector.tensor_copy(out=scc(CZi), in_=thc(10))
            tmp = pre.tile([P, 1], F32, tag="sctmp2")
            nc.vector.tensor_tensor(out=tmp[:], in0=thc(11), in1=thc(8),
                                    op=AO.subtract)
            nc.vector.tensor_tensor(out=tmp[:], in0=tmp[:], in1=thc(9),
                                    op=AO.subtract)
            nc.vector.tensor_tensor(out=tmp[:], in0=tmp[:], in1=thc(10),
                                    op=AO.subtract)
            nc.vector.tensor_scalar(out=scc(OZi), in0=tmp[:], scalar1=47.5,
                                    scalar2=47.5 + MZ, op0=AO.mult, op1=AO.add)
            nc.vector.tensor_scalar_add(out=scc(AXM1), in0=scc(AXi),
                                        scalar1=-1.0)
            nc.vector.tensor_scalar_mul(out=scc(SPXi), in0=scc(AXM1),
                                        scalar1=float(L - 1))
            nc.vector.tensor_scalar_mul(out=scc(SPYi), in0=scc(AYi),
                                        scalar1=float(L - 1))
            nc.vector.tensor_scalar_mul(out=scc(SPZi), in0=scc(AZi),
                                        scalar1=float(L - 1))

            # per-partition: p, jz = p div 40, m = p mod 40
            pidi = big.tile([P, 1], I32)
            nc.gpsimd.iota(pidi[:], [[0, 1]], base=0, channel_multiplier=1)
            pid = big.tile([P, 1], F32)
            nc.vector.tensor_copy(out=pid[:], in_=pidi[:])
            jzt = big.tile([P, 1], F32)
            jzi = big.tile([P, 1], I32)
            nc.vector.tensor_scalar(out=jzt[:], in0=pid[:], scalar1=-19.5,
                                    scalar2=1.0 / 40.0, op0=AO.add,
                                    op1=AO.mult)
            nc.vector.tensor_copy(out=jzi[:], in_=jzt[:])
            nc.vector.tensor_copy(out=jzt[:], in_=jzi[:])
            mt = big.tile([P, 1], F32)
            nc.vector.scalar_tensor_tensor(out=mt[:], in0=jzt[:], scalar=-40.0,
                                           in1=pid[:], op0=AO.mult, op1=AO.add)
            jzoff = big.tile([P, 1], F32)
            nc.vector.tensor_scalar_mul(out=jzoff[:], in0=jzt[:],
                                        scalar1=float(ZSTR))

            # sel [NP, TPG] bf16: (m == m')
            mio = big.tile([P, TPG], I32)
            nc.gpsimd.iota(mio[:], [[1, TPG]], base=0, channel_multiplier=0)
            miof = big.tile([P, TPG], F32)
            nc.vector.tensor_copy(out=miof[:], in_=mio[:])
            self_f = big.tile([P, TPG], F32)
            nc.vector.tensor_tensor(out=self_f[:], in0=fb(mt[:], [[0, TPG]]),
                                    in1=miof[:], op=AO.is_equal)
            sel = big.tile([P, TPG], BF16)
            nc.vector.tensor_copy(out=sel[:], in_=self_f[:])

            # iotas + templates (f16)
            def iotaf(n, tag):
                ti_ = pre.tile([P, n], I32, tag=tag + "i")
                nc.gpsimd.iota(ti_[:], [[1, n]], base=0, channel_multiplier=0)
                tf_ = pre.tile([P, n], F32, tag=tag + "f")
                nc.vector.tensor_copy(out=tf_[:], in_=ti_[:])
                return tf_

            wlf = iotaf(L, "wl")
            jyf = iotaf(KY, "jy")
            sxf = iotaf(KX, "sx")

            # zwlG[wl] = Az*wl - jz(p);  ywlG[jy,wl] = Ay*wl - jy;
            # xwlG[s,wl] = (Ax-1)*wl - s
            zwlG32 = pre.tile([P, L], F32, tag="zw32")
            nc.vector.scalar_tensor_tensor(
                out=zwlG32[:], in0=wlf[:], scalar=scc(AZi),
                in1=fb(jzt[:], [[0, L]]), op0=AO.mult, op1=AO.subtract)
            zwlG = big.tile([P, L], F16)
            nc.vector.tensor_copy(out=zwlG[:], in_=zwlG32[:])
            ywlG32 = pre.tile([P, KY * L], F32, tag="yw32")
            nc.vector.scalar_tensor_tensor(
                out=ywlG32[:], in0=fb(wlf[:], [[0, KY], [1, L]]),
                scalar=scc(AYi), in1=fb(jyf[:], [[1, KY], [0, L]]),
                op0=AO.mult, op1=AO.subtract)
            ywlG = big.tile([P, KY * L], F16)
            nc.vector.tensor_copy(out=ywlG[:], in_=ywlG32[:])
            xwlG32 = pre.tile([P, KX * L], F32, tag="xw32")
            nc.vector.scalar_tensor_tensor(
                out=xwlG32[:], in0=fb(wlf[:], [[0, KX], [1, L]]),
                scalar=scc(AXM1), in1=fb(sxf[:], [[1, KX], [0, L]]),
                op0=AO.mult, op1=AO.subtract)
            xwlG = big.tile([P, KX * L], F16)
            nc.vector.tensor_copy(out=xwlG[:], in_=xwlG32[:])

            # ---------- phase 3: per-task bases, residuals, indices ----------
            idxT = big.tile([P, NCOL], I32)
            rzT = big.tile([P, NCOL], F16)
            ryT = big.tile([P, NCOL], F16)
            rxT = big.tile([P, NCOL], F16)

            CHG = 512
            for c0 in range(0, NCOL, CHG):
                n = CHG
                gi = pre.tile([P, n], I32, tag="gi")
                nc.gpsimd.iota(gi[:], [[1, n]], base=c0, channel_multiplier=0)
                colf = pre.tile([P, n], F32, tag="colf")
                nc.vector.tensor_copy(out=colf[:], in_=gi[:])
                ti = pre.tile([P, n], I32, tag="ti")

                def fdiv(outt, int_, dv):
                    nc.vector.tensor_scalar(out=outt, in0=int_,
                                            scalar1=-(dv - 1.0) / 2.0,
                                            scalar2=1.0 / dv, op0=AO.add,
                                            op1=AO.mult)
                    nc.vector.tensor_copy(out=ti[:], in_=outt)
                    nc.vector.tensor_copy(out=outt, in_=ti[:])

                # cb = col div 16, cg = col mod 16
                cb = pre.tile([P, n], F32, tag="cb")
                fdiv(cb[:], colf[:], 16.0)
                cg = pre.tile([P, n], F32, tag="cg")
                nc.vector.scalar_tensor_tensor(out=cg[:], in0=cb[:],
                                               scalar=-16.0, in1=colf[:],
                                               op0=AO.mult, op1=AO.add)
                # r = cb*40 + m ; d = r div 160 ; h = r mod 160 ; w0 = cg*10
                rrow = pre.tile([P, n], F32, tag="rrow")
                nc.vector.tensor_scalar(out=rrow[:], in0=cb[:], scalar1=40.0,
                                        scalar2=mt[:], op0=AO.mult, op1=AO.add)
                dd = pre.tile([P, n], F32, tag="dd")
                fdiv(dd[:], rrow[:], 160.0)
                hh = pre.tile([P, n], F32, tag="hh")
                nc.vector.scalar_tensor_tensor(out=hh[:], in0=dd[:],
                                               scalar=-160.0, in1=rrow[:],
                                               op0=AO.mult, op1=AO.add)
                w0 = pre.tile([P, n], F32, tag="w0")
                nc.vector.tensor_scalar_mul(out=w0[:], in0=cg[:],
                                            scalar1=float(L))

                acci = pre.tile([P, n], F32, tag="acci")

                def base_resid(ai, bi, ci, oi, spi, bmax, resT, strd, first,
                               basef=None):
                    cin = pre.tile([P, n], F32, tag="cin")
                    nc.vector.tensor_scalar_mul(out=cin[:], in0=w0[:],
                                                scalar1=scc(ai))
                    nc.vector.scalar_tensor_tensor(
                        out=cin[:], in0=hh[:], scalar=scc(bi), in1=cin[:],
                        op0=AO.mult, op1=AO.add)
                    nc.vector.scalar_tensor_tensor(
                        out=cin[:], in0=dd[:], scalar=scc(ci), in1=cin[:],
                        op0=AO.mult, op1=AO.add)
                    nc.vector.tensor_scalar_add(out=cin[:], in0=cin[:],
                                                scalar1=scc(oi))
                    c9 = pre.tile([P, n], F32, tag="c9")
                    nc.vector.tensor_scalar_add(out=c9[:], in0=cin[:],
                                                scalar1=scc(spi))
                    nc.vector.tensor_tensor(out=c9[:], in0=c9[:], in1=cin[:],
                                            op=AO.min)
                    cbs = pre.tile([P, n], F32, tag="cbs")
                    nc.vector.tensor_scalar_add(out=cbs[:], in0=c9[:],
                                                scalar1=-0.499999)
                    nc.vector.tensor_copy(out=ti[:], in_=cbs[:])
                    nc.vector.tensor_copy(out=cbs[:], in_=ti[:])
                    nc.vector.tensor_scalar_max(out=cbs[:], in0=cbs[:],
                                                scalar1=0.0)
                    nc.vector.tensor_scalar_min(out=cbs[:], in0=cbs[:],
                                                scalar1=bmax)
                    res32 = pre.tile([P, n], F32, tag="res32")
                    nc.vector.tensor_tensor(out=res32[:], in0=cin[:],
                                            in1=cbs[:], op=AO.subtract)
                    nc.vector.tensor_copy(out=resT[:, c0:c0 + n],
                                          in_=res32[:])
                    if basef is not None:
                        nc.vector.tensor_copy(out=basef[:], in_=cbs[:])
                    elif first:
                        nc.vector.tensor_scalar_mul(out=acci[:], in0=cbs[:],
                                                    scalar1=float(strd))
                    else:
                        nc.vector.scalar_tensor_tensor(
                            out=acci[:], in0=cbs[:], scalar=float(strd),
                            in1=acci[:], op0=AO.mult, op1=AO.add)

                base_resid(AZi, BZi, CZi, OZi, SPZi, ZB_MAX, rzT, ZSTR, True)
                base_resid(AYi, BYi, CYi, OYi, SPYi, YB_MAX, ryT, RL, False)
                bxf = pre.tile([P, n], F32, tag="bxf")
                base_resid(AXi, BXi, CXi, OXi, SPXi, XB_MAX, rxT, 1, False,
                           basef=bxf)
                # strip = round((bx - 9)/19); xloc = bx - 19*strip
                stf = pre.tile([P, n], F32, tag="stf")
                nc.vector.tensor_scalar(out=stf[:], in0=bxf[:], scalar1=-9.0,
                                        scalar2=1.0 / 19.0, op0=AO.add,
                                        op1=AO.mult)
                nc.vector.tensor_copy(out=ti[:], in_=stf[:])
                nc.vector.tensor_copy(out=stf[:], in_=ti[:])
                # acci += strip*SSTR + (bx - 19*strip) -> acci + bx
                # note: acci currently = bz*ZSTR + by*RL
                nc.vector.tensor_tensor(out=acci[:], in0=acci[:], in1=bxf[:],
                                        op=AO.add)
                nc.vector.scalar_tensor_tensor(
                    out=acci[:], in0=stf[:], scalar=float(SSTR - 19),
                    in1=acci[:], op0=AO.mult, op1=AO.add)
                nc.vector.tensor_scalar_add(out=acci[:], in0=acci[:],
                                            scalar1=jzoff[:])
                nc.vector.tensor_copy(out=idxT[:, c0:c0 + n], in_=acci[:])

            # ---------- phase 4: main loop ----------
            pre_ctx.__exit__(None, None, None)
            gtp_ctx = tc.tile_pool(name="gtp", bufs=1)
            gtp = gtp_ctx.__enter__()
            work_ctx = tc.tile_pool(name="work", bufs=2)
            work = work_ctx.__enter__()

            GCL = G * C * L                   # 320
            GKYL = G * KY * L                 # 640
            GKXL = G * KX * L                 # 640
            GL = G * L                        # 160

            with tc.For_i(0, NBLK // UNR, 1, staggered_reset=True) as ib:
                gts = []
                for u_ in range(UNR):
                    idxfix = gtp.tile([P, G], I32, tag=f"idxfix{u_}")
                    nc.vector.tensor_copy(
                        out=idxfix[:NP, :],
                        in_=idxT[:NP, DynSlice((ib * UNR + u_) * G, G)])
                    gt = gtp.tile([P, G * STRM], BF16, tag=f"gt{u_}")
                    for g in range(G):
                        nc.gpsimd.indirect_dma_start(
                            out=gt[:NP, g * STRM:(g + 1) * STRM],
                            out_offset=None,
                            in_=pv[:],
                            in_offset=bass.IndirectOffsetOnAxis(
                                ap=idxfix[:NP, g:g + 1], axis=0))
                    gts.append(gt)
                for u_ in range(UNR):
                    gt = gts[u_]
                    col0 = (ib * UNR + u_) * G
                    rzs = rzT[:NP, DynSlice(col0, G)]
                    rys = ryT[:NP, DynSlice(col0, G)]
                    rxs = rxT[:NP, DynSlice(col0, G)]

                    # fields (f16) -> hats (bf16 via ACT)
                    zf = work.tile([P, GL], F16, tag="zf")
                    nc.vector.tensor_tensor(
                        out=zf[:NP, :], in0=fb(zwlG[:NP, :], [[0, G], [1, L]]),
                        in1=fb(rzs, [[1, G], [0, L]]), op=AO.add)
                    hz = work.tile([P, GL], BF16, tag="hz")
                    nc.scalar.activation(hz[:NP, :], zf[:NP, :], ACTF.Abs)
                    nc.scalar.activation(hz[:NP, :], hz[:NP, :], ACTF.Relu,
                                         bias=1.0, scale=-1.0)
                    yf = work.tile([P, GKYL], F16, tag="yf")
                    nc.vector.tensor_tensor(
                        out=yf[:NP, :],
                        in0=fb(ywlG[:NP, :], [[0, G], [1, KY * L]]),
                        in1=fb(rys, [[1, G], [0, KY * L]]), op=AO.add)
                    hy = work.tile([P, GKYL], BF16, tag="hy")
                    nc.scalar.activation(hy[:NP, :], yf[:NP, :], ACTF.Abs)
                    nc.scalar.activation(hy[:NP, :], hy[:NP, :], ACTF.Relu,
                                         bias=1.0, scale=-1.0)
                    xf = work.tile([P, GKXL], F16, tag="xf")
                    nc.vector.tensor_tensor(
                        out=xf[:NP, :],
                        in0=fb(xwlG[:NP, :], [[0, G], [1, KX * L]]),
                        in1=fb(rxs, [[1, G], [0, KX * L]]), op=AO.add)
                    gx = work.tile([P, GKXL], BF16, tag="gx")
                    nc.scalar.activation(gx[:NP, :], xf[:NP, :], ACTF.Abs)
                    nc.scalar.activation(gx[:NP, :], gx[:NP, :], ACTF.Relu,
                                         bias=1.0, scale=-1.0)

                    # hyz[g,jy,wl] = hy*hz ; duplicated for both c
                    hyz = work.tile([P, GKYL], BF16, tag="hyz")
                    nc.vector.tensor_tensor(
                        out=hyz[:NP, :], in0=hy[:NP, :],
                        in1=fb(hz[:NP, :], [[L, G], [0, KY], [1, L]]),
                        op=AO.mult)
                    hyzd = work.tile([P, C * GKYL], BF16, tag="hyzd")
                    nc.vector.tensor_copy(
                        out=hyzd[:NP, :],
                        in_=fb(hyz[:NP, :], [[0, C], [1, GKYL]]))

                    # x-stage: prodS[s][c,g,jy,wl] = gx[g,s,wl]*D[g,jy,c,wl+s]
                    prodS = []
                    for s in range(KX):
                        prS = work.tile([P, C * GKYL], BF16, tag=f"pr{s}")
                        prodS.append(prS)
                    for s in range(KX):
                        for c in range(C):
                            nc.vector.tensor_tensor(
                                out=fb(prodS[s][:NP, :],
                                       [[KY * L, G], [L, KY], [1, L]],
                                       extra_off=c * GKYL),
                                in0=fb(gt[:NP, :], [[STRM, G], [RL, KY],
                                                    [1, L]],
                                       extra_off=c * SW + s),
                                in1=fb(gx[:NP, :], [[KX * L, G], [0, KY],
                                                    [1, L]],
                                       extra_off=s * L),
                                op=AO.mult)
                    # tree adds -> tmpx in prodS[0]
                    nc.vector.tensor_tensor(out=prodS[0][:NP, :],
                                            in0=prodS[0][:NP, :],
                                            in1=prodS[1][:NP, :], op=AO.add)
                    nc.vector.tensor_tensor(out=prodS[2][:NP, :],
                                            in0=prodS[2][:NP, :],
                                            in1=prodS[3][:NP, :], op=AO.add)
                    nc.vector.tensor_tensor(out=prodS[0][:NP, :],
                                            in0=prodS[0][:NP, :],
                                            in1=prodS[2][:NP, :], op=AO.add)
                    nc.vector.tensor_tensor(out=prodS[0][:NP, :],
                                            in0=prodS[0][:NP, :],
                                            in1=prodS[4][:NP, :], op=AO.add)

                    # y-stage: yprod = tmpx * hyzd ; collapse jy
                    yprod = work.tile([P, C * GKYL], BF16, tag="yp")
                    nc.vector.tensor_tensor(out=yprod[:NP, :],
                                            in0=prodS[0][:NP, :],
                                            in1=hyzd[:NP, :], op=AO.mult)
                    # (c,g) merged dim stride KY*L, count C*G; jy slices
                    def jsl(t_, jy):
                        return fb(t_[:NP, :], [[KY * L, C * G], [1, L]],
                                  extra_off=jy * L)
                    t01 = work.tile([P, C * G * L], BF16, tag="t01")
                    nc.vector.tensor_tensor(out=t01[:NP, :],
                                            in0=jsl(yprod, 0),
                                            in1=jsl(yprod, 1), op=AO.add)
                    t23 = work.tile([P, C * G * L], BF16, tag="t23")
                    nc.vector.tensor_tensor(out=t23[:NP, :],
                                            in0=jsl(yprod, 2),
                                            in1=jsl(yprod, 3), op=AO.add)
                    yst = work.tile([P, C * G * L], BF16, tag="yst")
                    nc.vector.tensor_tensor(out=yst[:NP, :],
                                            in0=t01[:NP, :],
                                            in1=t23[:NP, :], op=AO.add)
                    # yst layout: (c, g, wl)

                    ps = pp.tile([TPG, C * G * L], F32, tag="ps")
                    nc.tensor.matmul(out=ps[:], lhsT=sel[:NP, :],
                                     rhs=yst[:NP, :], start=True, stop=True)
                    stg = work.tile([TPG, C * G * L], F32, tag="stg")
                    nc.vector.tensor_copy(out=stg[:], in_=ps[:])
                    for c in range(C):
                        dsl = out[0, DynSlice(
                            (ib * UNR + u_) * (TPG * G * L) + c * SRCEL,
                            TPG * G * L)]
                        dst = AP(dsl.tensor, dsl.offset,
                                 [[G * L, TPG], [1, G * L]])
                        nc.sync.dma_start(
                            out=dst,
                            in_=AP(stg[:].tensor, stg[:].offset + c * G * L,
                                   [list(stg[:].ap[0]), [1, G * L]]))
            work_ctx.__exit__(None, None, None)
            gtp_ctx.__exit__(None, None, None)

    nc.compile()
    return nc


def kernel(src, theta):
    if "prog" not in _CACHE:
        _CACHE["prog"] = _build_program()
    nc = _CACHE["prog"]
    in_maps = []
    for i in range(N):
        in_maps.append({
            "src": np.ascontiguousarray(src[i].reshape(C, SRCEL),
                                        dtype=np.float32),
            "theta": np.ascontiguousarray(theta[i].reshape(1, 12),
                                          dtype=np.float32),
        })
    res = run_bass_kernel_spmd(nc, in_maps, core_ids=list(range(N)))
    o = np.empty((N, C, D, H, W), dtype=np.float32)
    for i in range(N):
        o[i] = res.results[i]["out"].reshape(C, D, H, W)
    return o
